# revision 1
# baseline (speedup 1.0000x reference)
"""TRN2 Bass kernel for nn_CrossAttention_71287867179098.

Cross attention: out = softmax((x1@Wq) @ (x2@Wk)^T / sqrt(d)) @ (x2@Wv)
Shapes: x_1 [4096,1024], x_2 [4096,1024], W_* [1024,1024], out [4096,1024], fp32.

Sharding: query rows (x_1) split across 8 cores (512 rows each); x_2 and
weights replicated. Each core runs one-pass flash attention over kv chunks.

Algebra: the kv-side projections are folded out so the 4096-length kv axis is
touched by exactly one matmul on each side of the softmax:
  scores = (Q @ Wk^T) @ x2^T       (G = Q@Wk^T computed once, [512,1024])
  out    = ((P @ x2) @ Wv) / sums  (T = P@x2 accumulated in the flash loop)

Precision: PE matmuls run fp16 with 2-way hi/lo splits (A = Ah + Al, both
fp16; A@B ~= Ah@Bh + Al@Bh + Ah@Bl) on the x1->Q->G->scores chain, which the
near-one-hot softmax requires (top-2 score gaps can be < 0.01 while scores
span +-70000/32). P@x2 and T@Wv use single fp16. Accumulation is fp32 PSUM.
"""

import sys

sys.path.insert(0, "/opt/trn_rl_repo")

import numpy as np

import concourse.bass as bass
from concourse import bacc
import concourse.mybir as mybir
import concourse.tile as tile
from concourse.bass_utils import run_bass_kernel_spmd
from concourse.masks import make_identity

F32 = mybir.dt.float32
F16 = mybir.dt.float16
AX = mybir.AxisListType
ALU = mybir.AluOpType
ACTF = mybir.ActivationFunctionType

P = 128
D = 1024          # d_in == d_kq == d_v
CO = D // P       # contraction chunks (8)
NQ = 512          # query rows per core
QT = NQ // P      # query tiles per core (4)
NKV = 4096
CHUNK = 512       # kv rows per chunk
NCH = NKV // CHUNK
JO = CHUNK // P   # kv subtiles per chunk (4)
NCORES = 8
INV_SQRT_D = 1.0 / 32.0


def build_kernel() -> bass.Bass:
    nc = bacc.Bacc(target_bir_lowering=False)
    x1_d = nc.dram_tensor("x1s", [NQ, D], F32, kind="ExternalInput")
    x2_d = nc.dram_tensor("x2", [NKV, D], F32, kind="ExternalInput")
    wq_d = nc.dram_tensor("Wq", [D, D], F32, kind="ExternalInput")
    wk_d = nc.dram_tensor("Wk", [D, D], F32, kind="ExternalInput")
    wv_d = nc.dram_tensor("Wv", [D, D], F32, kind="ExternalInput")
    out_d = nc.dram_tensor("out", [NQ, D], F32, kind="ExternalOutput")

    with tile.TileContext(nc) as tc:
        with (
            tc.tile_pool(name="const", bufs=1) as constp,
            tc.tile_pool(name="persist", bufs=1) as persist,
            tc.tile_pool(name="stats", bufs=8) as stats,
            tc.tile_pool(name="psT", bufs=2, space="PSUM") as psT,
            tc.tile_pool(name="psTP", bufs=1, space="PSUM") as psTP,
            tc.tile_pool(name="psMM", bufs=2, space="PSUM") as psMM,
            tc.tile_pool(name="psS", bufs=2, space="PSUM") as psS,
            tc.tile_pool(name="psO", bufs=1, space="PSUM") as psO,
        ):
            ident32 = constp.tile([P, P], F32)
            make_identity(nc, ident32[:])
            ident16 = constp.tile([P, P], F16)
            make_identity(nc, ident16[:])

            # ---- persistent operands ----
            gt_h = persist.tile([P, CO, NQ], F16)   # G^T = Wk @ Q^T, hi
            gt_l = persist.tile([P, CO, NQ], F16)   # lo
            wv_h = persist.tile([P, CO, D], F16)
            t_acc = [persist.tile([P, D], F32, name=f"t_acc{q}") for q in range(QT)]
            m_cur = [stats.tile([P, 1], F32, tag="m", name=f"m0_{q}") for q in range(QT)]
            s_cur = [stats.tile([P, 1], F32, tag="s", name=f"s0_{q}") for q in range(QT)]
            for q in range(QT):
                nc.gpsimd.memset(t_acc[q][:], 0.0)
                nc.gpsimd.memset(m_cur[q][:], -1e30)
                nc.gpsimd.memset(s_cur[q][:], 0.0)

            # ---- phase 0: weights, Q^T, G^T (transient staging pool) ----
            with tc.tile_pool(name="stage", bufs=1) as stage:
                def load_split_w(dram, h_tile, l_tile):
                    wf = stage.tile([P, CO, D], F32, tag="wstage", bufs=1)
                    nc.sync.dma_start(wf[:], dram.rearrange("(co p) d -> p co d", p=P))
                    nc.scalar.activation(h_tile[:], wf[:], ACTF.Copy)
                    if l_tile is not None:
                        nc.vector.scalar_tensor_tensor(
                            l_tile[:], wf[:], 1.0, h_tile[:], ALU.mult, ALU.subtract
                        )
                    return wf

                wq_h = stage.tile([P, CO, D], F16, tag="wqh")
                wq_l = stage.tile([P, CO, D], F16, tag="wql")
                load_split_w(wq_d, wq_h, wq_l)

                # Wk: load fp32, transpose to WkT [d, c], split hi/lo fp16
                wkf = stage.tile([P, CO, D], F32, tag="wstage")
                nc.sync.dma_start(wkf[:], wk_d.rearrange("(co p) d -> p co d", p=P))
                wkt_h = stage.tile([P, CO, D], F16, tag="wkth")
                wkt_l = stage.tile([P, CO, D], F16, tag="wktl")
                for co in range(CO):
                    for dc in range(CO):
                        pst = psT.tile([P, P], F32, tag="pst")
                        nc.tensor.transpose(
                            pst[:], wkf[:, co, dc * P:(dc + 1) * P], ident32[:]
                        )
                        dst_h = wkt_h[:, dc, co * P:(co + 1) * P]
                        dst_l = wkt_l[:, dc, co * P:(co + 1) * P]
                        nc.scalar.activation(dst_h, pst[:], ACTF.Copy)
                        nc.vector.scalar_tensor_tensor(
                            dst_l, pst[:], 1.0, dst_h, ALU.mult, ALU.subtract
                        )

                load_split_w(wv_d, wv_h, None)  # phase-2 only; off critical path

                # x1 load, transpose, split -> x1T hi/lo [c, i] fp16
                x1c = stage.tile([P, QT, D], F32)
                nc.sync.dma_start(x1c[:], x1_d.rearrange("(io p) c -> p io c", p=P))
                x1t_h = stage.tile([P, CO, NQ], F16)
                x1t_l = stage.tile([P, CO, NQ], F16)
                for co in range(CO):
                    for io in range(QT):
                        pst = psT.tile([P, P], F32, tag="pst")
                        nc.tensor.transpose(
                            pst[:], x1c[:, io, co * P:(co + 1) * P], ident32[:]
                        )
                        dst_h = x1t_h[:, co, io * P:(io + 1) * P]
                        dst_l = x1t_l[:, co, io * P:(io + 1) * P]
                        nc.scalar.activation(dst_h, pst[:], ACTF.Copy)
                        nc.vector.scalar_tensor_tensor(
                            dst_l, pst[:], 1.0, dst_h, ALU.mult, ALU.subtract
                        )

                # Q^T [d, i]: lhsT=Wq[c,d] tiles, rhs=x1T[c,i]
                qt_h = stage.tile([P, CO, NQ], F16)
                qt_l = stage.tile([P, CO, NQ], F16)
                for dc in range(CO):
                    ps = psMM.tile([P, NQ], F32, tag="ps")
                    n = 0
                    for wt, xt in ((wq_h, x1t_h), (wq_l, x1t_h), (wq_h, x1t_l)):
                        for co in range(CO):
                            nc.tensor.matmul(
                                ps[:],
                                wt[:, co, dc * P:(dc + 1) * P],
                                xt[:, co, :],
                                start=(n == 0),
                                stop=(n == 23),
                            )
                            n += 1
                    nc.scalar.activation(qt_h[:, dc, :], ps[:], ACTF.Copy)
                    nc.vector.scalar_tensor_tensor(
                        qt_l[:, dc, :], ps[:], 1.0, qt_h[:, dc, :],
                        ALU.mult, ALU.subtract,
                    )

                # G^T [c, i] = Wk @ Q^T: lhsT=WkT[d,c] tiles, rhs=QT[d,i]
                for cc in range(CO):
                    ps = psMM.tile([P, NQ], F32, tag="ps")
                    n = 0
                    for wt, qa in ((wkt_h, qt_h), (wkt_l, qt_h), (wkt_h, qt_l)):
                        for dc in range(CO):
                            nc.tensor.matmul(
                                ps[:],
                                wt[:, dc, cc * P:(cc + 1) * P],
                                qa[:, dc, :],
                                start=(n == 0),
                                stop=(n == 23),
                            )
                            n += 1
                    nc.scalar.activation(gt_h[:, cc, :], ps[:], ACTF.Copy)
                    nc.vector.scalar_tensor_tensor(
                        gt_l[:, cc, :], ps[:], 1.0, gt_h[:, cc, :],
                        ALU.mult, ALU.subtract,
                    )

            # ---- phase 1: flash attention over kv chunks ----
            with (
                tc.tile_pool(name="x2pool", bufs=2) as x2pool,
                tc.tile_pool(name="x2t", bufs=2) as x2tpool,
                tc.tile_pool(name="pp", bufs=2) as ppool,
            ):
                def prepare(t):
                    """DMA chunk t, cast fp16 natural copy, transpose+split."""
                    x2c = x2pool.tile([P, JO, D], F32, tag="x2c", name=f"x2c_{t}")
                    src = x2_d[t * CHUNK:(t + 1) * CHUNK, :]
                    nc.sync.dma_start(
                        x2c[:], src.rearrange("(jo p) c -> p jo c", p=P)
                    )
                    # natural-layout fp16 copy (rhs of T += P^T-lhsT @ x2)
                    x2n = x2pool.tile([P, JO, D], F16, tag="x2n", name=f"x2n_{t}")
                    nc.scalar.activation(x2n[:], x2c[:], ACTF.Copy)

                    # transpose + split: x2T hi/lo [c, j] fp16
                    x2t_h = x2tpool.tile([P, CO, CHUNK], F16, tag="x2th", name=f"x2th_{t}")
                    x2t_l = x2tpool.tile([P, CO, CHUNK], F16, tag="x2tl", name=f"x2tl_{t}")
                    for co in range(CO):
                        for jo in range(JO):
                            pst = psT.tile([P, P], F32, tag="pst", name=f"pst_{t}_{co}_{jo}")
                            nc.tensor.transpose(
                                pst[:], x2c[:, jo, co * P:(co + 1) * P], ident32[:]
                            )
                            dst_h = x2t_h[:, co, jo * P:(jo + 1) * P]
                            dst_l = x2t_l[:, co, jo * P:(jo + 1) * P]
                            nc.scalar.activation(dst_h, pst[:], ACTF.Copy)
                            nc.vector.scalar_tensor_tensor(
                                dst_l, pst[:], 1.0, dst_h, ALU.mult, ALU.subtract
                            )
                    return x2n, x2t_h, x2t_l

                cur = prepare(0)
                for t in range(NCH):
                    x2n, x2t_h, x2t_l = cur
                    # scores + online softmax + T update per query tile;
                    # chunk t+1's transposes/casts are emitted after q=0 so
                    # they overlap q=1..3 matmuls on the other engines.
                    for q in range(QT):
                        ps_s = psS.tile([P, CHUNK], F32, tag="ps_s")
                        n = 0
                        for ga, xa in (
                            (gt_h, x2t_h), (gt_l, x2t_h), (gt_h, x2t_l)
                        ):
                            for cc in range(CO):
                                nc.tensor.matmul(
                                    ps_s[:],
                                    ga[:, cc, q * P:(q + 1) * P],
                                    xa[:, cc, :],
                                    start=(n == 0),
                                    stop=(n == 23),
                                )
                                n += 1

                        rm = stats.tile([P, 1], F32, tag="rm")
                        nc.vector.reduce_max(rm[:], ps_s[:], axis=AX.X)
                        m_new = stats.tile([P, 1], F32, tag="m")
                        nc.vector.tensor_tensor(
                            m_new[:], m_cur[q][:], rm[:], ALU.max
                        )
                        bias = stats.tile([P, 1], F32, tag="bias")
                        nc.vector.tensor_scalar_mul(bias[:], m_new[:], -INV_SQRT_D)
                        fsc = stats.tile([P, 1], F32, tag="fsc")
                        nc.scalar.activation(
                            fsc[:], m_cur[q][:], ACTF.Exp,
                            bias=bias[:], scale=INV_SQRT_D,
                        )
                        p_c = ppool.tile([P, CHUNK], F16, tag=f"p{q}")
                        rs = stats.tile([P, 1], F32, tag="rs")
                        nc.scalar.activation(
                            p_c[:], ps_s[:], ACTF.Exp,
                            bias=bias[:], scale=INV_SQRT_D, accum_out=rs[:],
                        )
                        s_new = stats.tile([P, 1], F32, tag="s")
                        nc.vector.scalar_tensor_tensor(
                            s_new[:], s_cur[q][:], fsc[:], rs[:], ALU.mult, ALU.add
                        )
                        m_cur[q] = m_new
                        s_cur[q] = s_new

                        # P^T tiles then T += P^T-lhsT @ x2n, rescaled by fsc
                        p_t = ppool.tile([P, JO, P], F16, tag=f"pt{q}")
                        for jt in range(JO):
                            pstp = psTP.tile([P, P], F16, tag="pstp")
                            nc.tensor.transpose(
                                pstp[:], p_c[:, jt * P:(jt + 1) * P], ident16[:]
                            )
                            nc.vector.tensor_copy(p_t[:, jt, :], pstp[:])
                        for dh in range(2):
                            ps_o = psO.tile([P, 512], F32, tag="ps_o")
                            for jt in range(JO):
                                nc.tensor.matmul(
                                    ps_o[:],
                                    p_t[:, jt, :],
                                    x2n[:, jt, dh * 512:(dh + 1) * 512],
                                    start=(jt == 0),
                                    stop=(jt == JO - 1),
                                )
                            dst = t_acc[q][:, dh * 512:(dh + 1) * 512]
                            nc.vector.scalar_tensor_tensor(
                                dst, dst, fsc[:], ps_o[:], ALU.mult, ALU.add
                            )
                        if q == 0 and t + 1 < NCH:
                            cur = prepare(t + 1)

            # ---- phase 2: normalize, O = (T/s) @ Wv, store ----
            with tc.tile_pool(name="outp", bufs=2) as outp:
                # Tn fp16 [i, c] per q-tile, then transpose to TT [c, i]
                tt = outp.tile([P, CO, NQ], F16, name="tt", bufs=1)
                for q in range(QT):
                    rcp = stats.tile([P, 1], F32, tag="rcp")
                    nc.vector.reciprocal(rcp[:], s_cur[q][:])
                    tn = outp.tile([P, D], F16, tag="tn")
                    nc.scalar.activation(
                        tn[:], t_acc[q][:], ACTF.Copy, scale=rcp[:]
                    )
                    for cc in range(CO):
                        pstp = psTP.tile([P, P], F16, tag="pstp")
                        nc.tensor.transpose(
                            pstp[:], tn[:, cc * P:(cc + 1) * P], ident16[:]
                        )
                        nc.vector.tensor_copy(
                            tt[:, cc, q * P:(q + 1) * P], pstp[:]
                        )

                out_ap = out_d.rearrange("(qo p) d -> p qo d", p=P)
                for q in range(QT):
                    o_sb = outp.tile([P, D], F32, tag="osb")
                    for dh in range(2):
                        ps = psMM.tile([P, 512], F32, tag="ps")
                        for cc in range(CO):
                            nc.tensor.matmul(
                                ps[:],
                                tt[:, cc, q * P:(q + 1) * P],
                                wv_h[:, cc, dh * 512:(dh + 1) * 512],
                                start=(cc == 0),
                                stop=(cc == CO - 1),
                            )
                        nc.vector.tensor_copy(o_sb[:, dh * 512:(dh + 1) * 512], ps[:])
                    nc.sync.dma_start(out_ap[:, q, :], o_sb[:])

    nc.compile()
    return nc


_NC_CACHE = None


def _get_nc():
    global _NC_CACHE
    if _NC_CACHE is None:
        _NC_CACHE = build_kernel()
    return _NC_CACHE


def _run(inputs, trace=False):
    """Returns (output [4096,1024] f32, exec_time_ns or None, results obj)."""
    x1 = np.ascontiguousarray(np.asarray(inputs["x_1"], dtype=np.float32))
    x2 = np.ascontiguousarray(np.asarray(inputs["x_2"], dtype=np.float32))
    wq = np.ascontiguousarray(np.asarray(inputs["W_query"], dtype=np.float32))
    wk = np.ascontiguousarray(np.asarray(inputs["W_key"], dtype=np.float32))
    wv = np.ascontiguousarray(np.asarray(inputs["W_value"], dtype=np.float32))

    nc = _get_nc()
    in_maps = [
        {
            "x1s": x1[c * NQ:(c + 1) * NQ],
            "x2": x2,
            "Wq": wq,
            "Wk": wk,
            "Wv": wv,
        }
        for c in range(NCORES)
    ]
    br = run_bass_kernel_spmd(nc, in_maps, list(range(NCORES)), trace=trace)
    out = np.concatenate([br.results[c]["out"] for c in range(NCORES)], axis=0)
    return out.astype(np.float32), br.exec_time_ns, br


def kernel(**inputs) -> np.ndarray:
    out, _, _ = _run(inputs)
    return out

